# revision 22
# baseline (speedup 1.0000x reference)
"""Cost-volume kernel for Trainium2 (8 NeuronCores, batch-parallel).

out[b, k, h, w] = (1/(C*81)) * sum_c x[b,c,h,w] * warped[b,c,h+di,w+dj]
for the 81 offsets (di,dj) in [-4,4]^2 (zero-padded), B=8 -> one batch
element per core.

Device-side algorithm (per core), v2 "group-banded dump":
  - the image is tiled into 16x8 x-tiles (8x32 = 256 tiles), pixel m =
    r*8 + q. One TensorE matmul per tile: lhsT = x-tile [C, 128]
    (host-prepped contiguous), rhs = 24x16 window view into the SBUF-
    resident zero-padded warped image [C, 136, 264] (2-free-dim AP, no
    halo duplication) -> PSUM [128, 384].
  - PSUM drains (x1/(C*81), cast bf16) to SBUF in groups of 4 tiles
    (ACT/DVE alternating), 2 groups per dump tile [128, 8*384].
  - Stores exploit that for a 16-partition group g (rows r in {2g,2g+1})
    the union of needed window columns is the CONTIGUOUS run
    [32g, 32g+160): 8 partition-subrange DMAs per dump tile ship only
    160 els/pixel (vs 384), a legal 3-dim access pattern.
  - Host unpacks the 160-wide bands into [81, H, W] with one as_strided
    view (value for (di,dj) at band offset (e+di)*16 + q+dj, e = r&1).
"""

import numpy as np

B = 8
C, H, W = 128, 128, 256
R = 4
K = 2 * R + 1  # 9
NOFF = K * K  # 81
TH, TW = 16, 8  # x-tile shape (M = TH*TW = 128)
NTH, NTW = H // TH, W // TW  # 8 x 32 tiles
NT = NTH * NTW  # 256
WR, WC = TH + 2 * R, TW + 2 * R  # window 24 x 16
N = WR * WC  # 384
PH, PW = H + 2 * R, W + 2 * R  # padded warped image 136 x 264
BAND = 160  # per-group banded store width: (1+8)*16 + 7 + 8 + 1
GD = 2  # tiles per PSUM drain group
TPD = 32  # tiles per dump tile (one full band row -> 8 stores per band)
ND = NT // TPD  # 8 dump tiles
SCALE = 1.0 / (C * NOFF)

PRECISION = "bf16"

_CACHE = {}


def _build_module(n_cores, precision):
    import concourse.bacc as bacc
    import concourse.mybir as mybir
    import concourse.tile as tile

    dt = mybir.dt.float32 if precision == "f32" else mybir.dt.bfloat16
    f32 = mybir.dt.float32
    BANK = 512  # fp32 elements per PSUM bank

    nc = bacc.Bacc(
        "TRN2", target_bir_lowering=False, debug=False, num_devices=n_cores
    )
    x_d = nc.dram_tensor("x", [C, NT * 128], dt, kind="ExternalInput").ap()
    w_d = nc.dram_tensor("warped", [C, PH * PW], dt, kind="ExternalInput").ap()
    # per band: group g ships window rows [2g, 2g+10) x (32 tiles x 16 cols)
    out_d = nc.dram_tensor(
        "dump", [ND, TH // 2, 16, (K + 1) * TPD * WC], dt, kind="ExternalOutput"
    ).ap()

    with tile.TileContext(nc) as tc:
        with (
            tc.tile_pool(name="ximg", bufs=1) as x_pool,
            tc.tile_pool(name="wimg", bufs=1) as w_pool,
            tc.tile_pool(name="dump", bufs=3) as dump_pool,
            tc.tile_pool(name="psum", bufs=4, space="PSUM") as psum_pool,
        ):
            xt = x_pool.tile([C, NT * 128], dt)
            wt = w_pool.tile([C, PH * PW], dt)
            # chunked loads (all on the SP HWDGE ring; stores ride ACT/
            # SWDGE so ring FIFOs never put stores behind loads). Chunk
            # ith exactly completes the rows band ith's matmuls read.
            for ith in range(NTH):
                nc.sync.dma_start(
                    out=xt[:, ith * NTW * 128 : (ith + 1) * NTW * 128],
                    in_=x_d[:, ith * NTW * 128 : (ith + 1) * NTW * 128],
                )
                r0 = 0 if ith == 0 else TH * ith + 2 * R
                r1 = TH * ith + WR
                nc.sync.dma_start(
                    out=wt[:, r0 * PW : r1 * PW], in_=w_d[:, r0 * PW : r1 * PW]
                )

            wv = wt[:].rearrange("c (r q) -> c r q", r=PH)
            store_eng = [nc.gpsimd, nc.scalar]
            t = 0
            ps = None
            db = None
            for ith in range(NTH):
                for itw in range(NTW):
                    j = t % GD
                    if j == 0:
                        ps = psum_pool.tile([128, GD * BANK], f32)
                    if t % TPD == 0:
                        db = dump_pool.tile([128, TPD * N], dt)
                    lhsT = xt[:, t * 128 : (t + 1) * 128]
                    rhs = wv[:, TH * ith : TH * ith + WR, TW * itw : TW * itw + WC]
                    nc.tensor.matmul(
                        ps[:, j * BANK : j * BANK + N], lhsT, rhs,
                        start=True, stop=True,
                    )
                    t += 1
                    if j == GD - 1:
                        tp = (t // GD - 1) % (TPD // GD)  # pair slot in dump tile
                        # PSUM pair -> row-major-across-tiles layout:
                        # db[p, r*512 + (2*tp+jl)*16 + c]
                        src = (
                            ps[:]
                            .rearrange("p (j x) -> p j x", j=GD)[:, :, 0:N]
                            .rearrange("p j (r c) -> p r j c", c=WC)
                        )
                        dst = (
                            db[:]
                            .rearrange("p (r u) -> p r u", r=WR)[
                                :, :, GD * WC * tp : GD * WC * (tp + 1)
                            ]
                            .rearrange("p r (j c) -> p r j c", j=GD)
                        )
                        if tp % 2 == 0:
                            nc.scalar.mul(dst, src, SCALE)
                        else:
                            nc.vector.tensor_scalar_mul(dst, src, SCALE)
                    if t % TPD == 0:
                        d = t // TPD - 1
                        U = TPD * WC  # 512 els per dump-tile row
                        for grp in range(TH // 2):
                            eng = store_eng[(d * (TH // 2) + grp) % len(store_eng)]
                            # rows [2g, 2g+10) are CONTIGUOUS in the dump tile:
                            # one 10KB run per partition
                            eng.dma_start(
                                out=out_d[d, grp],
                                in_=db[
                                    16 * grp : 16 * grp + 16,
                                    2 * grp * U : (2 * grp + K + 1) * U,
                                ],
                            )
            assert t == NT

    nc.compile()
    return nc


def _host_prep(x_b, warped_b):
    """x: [C,H,W] -> per-tile-contiguous [C, NT*128] (band-major tiles);
    warped: [C,H,W] -> zero-padded [C, PH*PW]."""
    xs = np.ascontiguousarray(
        x_b.reshape(C, NTH, TH, NTW, TW).transpose(0, 1, 3, 2, 4)
    ).reshape(C, NT * 128)
    wp = np.zeros((C, PH, PW), dtype=warped_b.dtype)
    wp[:, R : R + H, R : R + W] = warped_b
    return xs, wp.reshape(C, PH * PW)


def _extract(dump):
    """[ND, 8, 16, TPD*BAND] -> [81, H, W] via one as_strided view.

    dump[ith, g, p, (e+di)*512 + itw*16 + (q+dj)] = scaled dot(x[:, h, w],
    warped[:, h+di-R, w+dj-R]) where r = 2g + e, e = p // 8, q = p % 8,
    h = TH*ith + r, w = TW*itw + q.
    """
    dmp = np.ascontiguousarray(dump).reshape(ND, TH // 2, 16, (K + 1) * TPD * WC)
    s_d, s_g, s_p, s_j = dmp.strides
    view = np.lib.stride_tricks.as_strided(
        dmp,
        shape=(K, K, NTH, TH // 2, 2, NTW, TW),
        strides=(
            TPD * WC * s_j,            # di
            s_j,                       # dj
            s_d,                       # ith
            s_g,                       # g
            8 * s_p + TPD * WC * s_j,  # e
            WC * s_j,                  # itw
            s_p + s_j,                 # q
        ),
    )
    return view.reshape(NOFF, H, W).astype(np.float32)


def kernel(x, warped):
    from concourse import bass_utils

    x = np.asarray(x, dtype=np.float32)
    warped = np.asarray(warped, dtype=np.float32)
    assert x.shape == (B, C, H, W) and warped.shape == (B, C, H, W)

    if PRECISION == "bf16":
        import ml_dtypes

        x = x.astype(ml_dtypes.bfloat16)
        warped = warped.astype(ml_dtypes.bfloat16)

    key = PRECISION
    if key not in _CACHE:
        _CACHE[key] = _build_module(B, PRECISION)
    nc = _CACHE[key]

    in_maps = []
    for b in range(B):
        xs, ws = _host_prep(x[b], warped[b])
        in_maps.append({"x": xs, "warped": ws})
    res = bass_utils.run_bass_kernel_spmd(nc, in_maps, core_ids=list(range(B)))
    global LAST_RESULTS
    LAST_RESULTS = res
    out = np.empty((B, NOFF, H, W), dtype=np.float32)
    for b in range(B):
        out[b] = _extract(res.results[b]["dump"])
    return out
